# revision 24
# baseline (speedup 1.0000x reference)
"""Trainium2 SPMD kernel for: y = BatchNorm1d(x @ sign(w).T + bias) * gamma + beta.

Sharding: data-parallel over the batch dim across 8 NeuronCores; the
weight is replicated.  BatchNorm batch statistics are produced with an
on-device AllReduce of per-shard (sum_y, sum_y2).

Math notes:
  - The linear bias cancels inside BatchNorm (y - mean), so it is never
    applied on device.
  - sign(w) in {-1,+1} is computed as (w >= 0) - 0.5 in {-0.5,+0.5}; the
    resulting global scale of 0.5 also cancels in BatchNorm except in the
    epsilon, which is compensated with eps/4.
  - Matmul runs in bf16 (weights +-0.5 are exact; x rounding gives
    ~2e-3 relative error, far inside tolerance).  fp32->bf16 conversion
    happens inside the DMA (SWDGE casting DMA), not on compute engines.
"""

import os
import sys

sys.path.insert(0, "/opt/trn_rl_repo")

import numpy as np

import concourse.bacc as bacc
import concourse.mybir as mybir
import concourse.tile as tile
from concourse import bass_utils

N_CORES = 8
B_TOT = 16384
D_IN = 2048
D_OUT = 1024
B_SH = B_TOT // N_CORES           # 2048 rows per core
KT = D_IN // 128                  # 16 contraction tiles
BT = B_SH // 128                  # 16 batch tiles per core
BN_EPS = 1e-5

F32 = mybir.dt.float32
BF16 = mybir.dt.bfloat16

# every AR_WARM-th batch tile fires a dummy all-reduce; 0 disables
AR_WARM = int(os.environ.get("KERNEL_AR_WARM", "3"))


def build_kernel():
    nc = bacc.Bacc("TRN2", target_bir_lowering=False, debug=False,
                   num_devices=N_CORES)

    xt = nc.dram_tensor("xt", [D_IN, B_SH], F32, kind="ExternalInput")
    wt = nc.dram_tensor("wt", [D_IN, D_OUT], F32, kind="ExternalInput")
    gamma = nc.dram_tensor("gamma", [1, D_OUT], F32, kind="ExternalInput")
    beta = nc.dram_tensor("beta", [1, D_OUT], F32, kind="ExternalInput")
    out = nc.dram_tensor("out", [B_SH, D_OUT], F32, kind="ExternalOutput")

    with tile.TileContext(nc) as tc:
        with tc.tile_pool(name="persist", bufs=1) as persist, \
             tc.tile_pool(name="xin", bufs=4) as xin_pool, \
             tc.tile_pool(name="work", bufs=3) as work_pool, \
             tc.tile_pool(name="stage", bufs=3) as stage_pool, \
             tc.tile_pool(name="psum", bufs=2, space="PSUM") as psum_pool, \
             tc.tile_pool(name="spsum", bufs=1, space="PSUM") as spsum_pool, \
             tc.tile_pool(name="dram", bufs=1, space="DRAM") as dram:

            ones = persist.tile([128, 1], BF16)
            nc.vector.memset(ones[:], 1.0)

            # ---- weights: per-stripe tiles so matmuls depend only on their
            # ---- own stripe; loads split across both HWDGE rails ----
            wbs = [persist.tile([128, D_OUT], BF16, name=f"wb{i}")
                   for i in range(KT)]
            xb0 = xin_pool.tile([128, KT * 128], BF16, name="xb0", tag="xb")
            # start the first x block as early as possible (host supplies
            # xt in bt-major blocks: row bt*128+p, col kt*128+b -> the load
            # is a plain contiguous 2D slice)
            nc.gpsimd.dma_start(xb0[:], xt[0:128, :])
            for it in range(KT):
                # HWDGE f32 load; the sign op below does the bf16 conversion
                wtmp = work_pool.tile([128, D_OUT], F32, name=f"wtmp{it}",
                                      tag=f"wtmp{it % 4}")
                eng = nc.sync if it % 2 == 0 else nc.scalar
                eng.dma_start(wtmp[:], wt[it * 128:(it + 1) * 128, :])
                # (w >= 0) - 0.5  ->  {+0.5, -0.5}
                nc.vector.tensor_scalar(
                    out=wbs[it][:],
                    in0=wtmp[:],
                    scalar1=0.0, scalar2=0.5,
                    op0=mybir.AluOpType.is_ge,
                    op1=mybir.AluOpType.subtract,
                )

            # ---- persistent y (bf16) and stats accumulators (PSUM) ----
            y_all = persist.tile([128, BT * D_OUT], BF16)    # [b%128, (bt, o)]
            sy = spsum_pool.tile([1, D_OUT], F32)            # sum(y)   per o
            sy2 = spsum_pool.tile([1, D_OUT], F32)           # sum(y^2) per o

            # ---- main loop over batch tiles ----
            # stats matmuls run one bt behind the main matmuls so the tensor
            # engine never waits on the DVE/ACT producers of their inputs.
            pending_stats = []

            def flush_stats(first, last):
                for ot, yslice, y2t in pending_stats:
                    nc.tensor.matmul(sy[0:1, ot * 512:ot * 512 + 512],
                                     ones[:], yslice,
                                     start=first, stop=last)
                    nc.tensor.matmul(sy2[0:1, ot * 512:ot * 512 + 512],
                                     ones[:], y2t[:],
                                     start=first, stop=last)
                pending_stats.clear()

            for bt in range(BT):
                if bt == 0:
                    xb = xb0
                else:
                    xb = xin_pool.tile([128, KT * 128], BF16, name=f"xb{bt}",
                                       tag="xb")
                    nc.gpsimd.dma_start(
                        xb[:], xt[bt * 128:(bt + 1) * 128, :])
                new_stats = []
                for ot in range(2):
                    acc = psum_pool.tile([128, 512], F32, name=f"acc{bt}_{ot}",
                                         tag=f"acc{ot}")
                    for it in range(KT):
                        nc.tensor.matmul(
                            acc[:],
                            xb[:, it * 128:(it + 1) * 128],
                            wbs[it][:, ot * 512:ot * 512 + 512],
                            start=(it == 0), stop=(it == KT - 1),
                        )
                    yslice = y_all[:, bt * D_OUT + ot * 512:
                                   bt * D_OUT + ot * 512 + 512]
                    nc.vector.tensor_copy(yslice, acc[:])
                    y2t = work_pool.tile([128, 512], BF16, name=f"y2_{bt}_{ot}",
                                         tag=f"y2_{ot}")
                    nc.scalar.activation(y2t[:], acc[:],
                                         mybir.ActivationFunctionType.Square)
                    new_stats.append((ot, yslice, y2t))
                flush_stats(bt == 1, False)
                pending_stats.extend(new_stats)

                if AR_WARM and bt % AR_WARM == AR_WARM - 1 and bt < BT - 1:
                    # Paced dummy all-reduces keep the collective engine awake
                    # so the real stats all-reduce is cheap.  Nothing ever
                    # waits on their outputs.
                    wi = dram.tile([1, 8], F32, name=f"warm_i{bt}",
                                   tag=f"warm_i{bt}")
                    wo = dram.tile([1, 8], F32, name=f"warm_o{bt}",
                                   tag=f"warm_o{bt}")
                    nc.gpsimd.dma_start(
                        wi[:], y_all[0:1, max(bt - 2, 0) * D_OUT:
                                     max(bt - 2, 0) * D_OUT + 8])
                    nc.gpsimd.collective_compute(
                        "AllReduce", mybir.AluOpType.add,
                        replica_groups=[list(range(N_CORES))],
                        ins=[wi.opt()], outs=[wo.opt()],
                    )
            flush_stats(False, True)

            # ---- global stats all-reduce ----
            stats = persist.tile([1, 2 * D_OUT], F32)
            nc.vector.tensor_copy(stats[0:1, 0:D_OUT], sy[:])
            nc.vector.tensor_copy(stats[0:1, D_OUT:2 * D_OUT], sy2[:])
            cbi = dram.tile([1, 2 * D_OUT], F32)
            cbo = dram.tile([1, 2 * D_OUT], F32)
            nc.gpsimd.dma_start(cbi[:], stats[:])
            nc.gpsimd.collective_compute(
                "AllReduce", mybir.AluOpType.add,
                replica_groups=[list(range(N_CORES))],
                ins=[cbi.opt()], outs=[cbo.opt()],
            )
            # coefficient math in [128, 8] layout (o = p*8 + j) so all 128
            # DVE lanes work instead of one
            PJ = 2 * D_OUT // 128     # 16 = [a-half 8 | c-half 8]
            gs8 = persist.tile([128, PJ], F32)
            nc.sync.dma_start(gs8[:, 0:8], cbo[0:1, 0:D_OUT].rearrange(
                "a (p j) -> (a p) j", p=128))
            nc.sync.dma_start(gs8[:, 8:16], cbo[0:1, D_OUT:2 * D_OUT].rearrange(
                "a (p j) -> (a p) j", p=128))

            # ---- coefficients: a = gamma/sqrt(var+eps/4), c = beta - mean*a
            gam = persist.tile([128, 8], F32)
            bet = persist.tile([128, 8], F32)
            nc.sync.dma_start(gam[:], gamma.rearrange("a (p j) -> (a p) j",
                                                      p=128))
            nc.sync.dma_start(bet[:], beta.rearrange("a (p j) -> (a p) j",
                                                     p=128))

            coef = persist.tile([128, PJ], F32)   # [a(8) | c(8)] per partition
            mean = persist.tile([128, 8], F32)
            var = persist.tile([128, 8], F32)
            m2 = persist.tile([128, 8], F32)
            inv = persist.tile([128, 8], F32)
            nc.vector.tensor_scalar_mul(mean[:], gs8[:, 0:8], 1.0 / B_TOT)
            nc.vector.tensor_scalar_mul(var[:], gs8[:, 8:16], 1.0 / B_TOT)
            nc.vector.tensor_tensor(out=m2[:], in0=mean[:], in1=mean[:],
                                    op=mybir.AluOpType.mult)
            nc.vector.tensor_tensor(out=var[:], in0=var[:], in1=m2[:],
                                    op=mybir.AluOpType.subtract)
            nc.vector.tensor_scalar_add(var[:], var[:], BN_EPS / 4.0)
            nc.scalar.activation(inv[:], var[:],
                                 mybir.ActivationFunctionType.Sqrt)
            nc.vector.reciprocal(inv[:], inv[:])
            nc.vector.tensor_tensor(out=coef[:, 0:8], in0=gam[:],
                                    in1=inv[:], op=mybir.AluOpType.mult)
            # c = beta - mean * a
            tmp_ma = persist.tile([128, 8], F32)
            nc.vector.tensor_tensor(out=tmp_ma[:], in0=mean[:],
                                    in1=coef[:, 0:8],
                                    op=mybir.AluOpType.mult)
            nc.vector.tensor_tensor(out=coef[:, 8:16],
                                    in0=bet[:], in1=tmp_ma[:],
                                    op=mybir.AluOpType.subtract)

            # ---- broadcast coefficients to all 128 partitions ----
            # coef[p, 0:8] holds a[p*8 : p*8+8]; write back to DRAM flat,
            # then broadcast-read
            coefd = dram.tile([1, 2 * D_OUT], F32)
            nc.sync.dma_start(coefd[0:1, :].rearrange(
                "a (half p j) -> (a p) half j", p=128, half=2), coef[:]
                .rearrange("p (half j) -> p half j", half=2))
            REP = 4                                   # bt blocks per DVE op
            ab = persist.tile([128, D_OUT], BF16)
            cb = persist.tile([128, D_OUT], BF16)
            nc.gpsimd.dma_start(ab[:], coefd[0:1, 0:D_OUT]
                                .partition_broadcast(128))
            nc.gpsimd.dma_start(cb[:], coefd[0:1, D_OUT:2 * D_OUT]
                                .partition_broadcast(128))

            # ---- normalize and write out (all bf16 for DVE 2x mode;
            # ---- the store DMA casts bf16 -> f32) ----
            for c in range(BT // REP):
                w_ = REP * D_OUT
                tmp = work_pool.tile([128, w_], BF16, name=f"nt{c}",
                                     tag="ntmp")
                nc.vector.tensor_tensor(
                    out=tmp[:].rearrange("p (r o) -> p r o", r=REP),
                    in0=y_all[:, c * w_:(c + 1) * w_]
                    .rearrange("p (r o) -> p r o", r=REP),
                    in1=ab[:].unsqueeze(1).broadcast_to((128, REP, D_OUT)),
                    op=mybir.AluOpType.mult)
                stg = stage_pool.tile([128, w_], BF16, name=f"stg{c}",
                                      tag="stg")
                nc.vector.tensor_tensor(
                    out=stg[:].rearrange("p (r o) -> p r o", r=REP),
                    in0=tmp[:].rearrange("p (r o) -> p r o", r=REP),
                    in1=cb[:].unsqueeze(1).broadcast_to((128, REP, D_OUT)),
                    op=mybir.AluOpType.add)
                nc.gpsimd.dma_start(
                    out.rearrange("(c r p) o -> c p r o",
                                  r=REP, p=128)[c, :, :, :],
                    stg[:].rearrange("p (r o) -> p r o", r=REP))

    nc.compile()
    return nc


_NC_CACHE = None


def kernel(x, weight, bias, gamma, beta):
    global _NC_CACHE
    if _NC_CACHE is None:
        _NC_CACHE = build_kernel()
    nc = _NC_CACHE

    x = np.asarray(x, dtype=np.float32)
    weight = np.asarray(weight, dtype=np.float32)
    gamma = np.asarray(gamma, dtype=np.float32).reshape(1, D_OUT)
    beta = np.asarray(beta, dtype=np.float32).reshape(1, D_OUT)

    wt = np.ascontiguousarray(weight.T)
    in_maps = []
    for i in range(N_CORES):
        shard = x[i * B_SH:(i + 1) * B_SH]
        blk = shard.reshape(BT, 128, KT, 128).transpose(0, 3, 2, 1)
        in_maps.append({
            "xt": np.ascontiguousarray(blk).reshape(BT * 128, KT * 128),
            "wt": wt,
            "gamma": gamma,
            "beta": beta,
        })

    res = bass_utils.run_bass_kernel_spmd(
        nc, in_maps, core_ids=list(range(N_CORES)),
        trace=bool(int(os.environ.get("KERNEL_TRACE", "0"))),
    )
    kernel.last_results = res
    return np.concatenate([res.results[i]["out"] for i in range(N_CORES)],
                          axis=0)


# revision 25
# speedup vs baseline: 1.0469x; 1.0469x over previous
"""Trainium2 SPMD kernel for: y = BatchNorm1d(x @ sign(w).T + bias) * gamma + beta.

Sharding: data-parallel over the batch dim across 8 NeuronCores; the
weight is replicated.  BatchNorm batch statistics are produced with an
on-device AllReduce of per-shard (sum_y, sum_y2).

Math notes:
  - The linear bias cancels inside BatchNorm (y - mean), so it is never
    applied on device.
  - sign(w) in {-1,+1} is computed as (w >= 0) - 0.5 in {-0.5,+0.5}; the
    resulting global scale of 0.5 also cancels in BatchNorm except in the
    epsilon, which is compensated with eps/4.
  - Matmul runs in bf16 (weights +-0.5 are exact; x rounding gives
    ~2e-3 relative error, far inside tolerance).  fp32->bf16 conversion
    happens inside the DMA (SWDGE casting DMA), not on compute engines.
"""

import os
import sys

sys.path.insert(0, "/opt/trn_rl_repo")

import numpy as np

import concourse.bacc as bacc
import concourse.mybir as mybir
import concourse.tile as tile
from concourse import bass_utils

N_CORES = 8
B_TOT = 16384
D_IN = 2048
D_OUT = 1024
B_SH = B_TOT // N_CORES           # 2048 rows per core
KT = D_IN // 128                  # 16 contraction tiles
BT = B_SH // 128                  # 16 batch tiles per core
BN_EPS = 1e-5

F32 = mybir.dt.float32
BF16 = mybir.dt.bfloat16

# every AR_WARM-th batch tile fires a dummy all-reduce; 0 disables
AR_WARM = int(os.environ.get("KERNEL_AR_WARM", "3"))


def build_kernel():
    nc = bacc.Bacc("TRN2", target_bir_lowering=False, debug=False,
                   num_devices=N_CORES)

    xt = nc.dram_tensor("xt", [D_IN, B_SH], F32, kind="ExternalInput")
    wt = nc.dram_tensor("wt", [D_IN, D_OUT], F32, kind="ExternalInput")
    gamma = nc.dram_tensor("gamma", [1, D_OUT], F32, kind="ExternalInput")
    beta = nc.dram_tensor("beta", [1, D_OUT], F32, kind="ExternalInput")
    out = nc.dram_tensor("out", [B_SH, D_OUT], F32, kind="ExternalOutput")

    with tile.TileContext(nc) as tc:
        with tc.tile_pool(name="persist", bufs=1) as persist, \
             tc.tile_pool(name="xin", bufs=4) as xin_pool, \
             tc.tile_pool(name="work", bufs=3) as work_pool, \
             tc.tile_pool(name="stage", bufs=3) as stage_pool, \
             tc.tile_pool(name="psum", bufs=2, space="PSUM") as psum_pool, \
             tc.tile_pool(name="spsum", bufs=1, space="PSUM") as spsum_pool, \
             tc.tile_pool(name="dram", bufs=1, space="DRAM") as dram:

            ones = persist.tile([128, 1], BF16)
            nc.vector.memset(ones[:], 1.0)

            # ---- weights: per-stripe tiles so matmuls depend only on their
            # ---- own stripe; loads split across both HWDGE rails ----
            wbs = [persist.tile([128, D_OUT], BF16, name=f"wb{i}")
                   for i in range(KT)]
            xb0 = xin_pool.tile([128, KT * 128], BF16, name="xb0", tag="xb")
            # start the first x block as early as possible (host supplies
            # xt in bt-major blocks: row bt*128+p, col kt*128+b -> the load
            # is a plain contiguous 2D slice)
            nc.gpsimd.dma_start(xb0[:], xt[0:128, :])
            for it in range(KT):
                # HWDGE f32 load; the sign op below does the bf16 conversion
                wtmp = work_pool.tile([128, D_OUT], F32, name=f"wtmp{it}",
                                      tag=f"wtmp{it % 4}")
                eng = nc.sync if it % 2 == 0 else nc.scalar
                eng.dma_start(wtmp[:], wt[it * 128:(it + 1) * 128, :])
                # (w >= 0) - 0.5  ->  {+0.5, -0.5}
                nc.vector.tensor_scalar(
                    out=wbs[it][:],
                    in0=wtmp[:],
                    scalar1=0.0, scalar2=0.5,
                    op0=mybir.AluOpType.is_ge,
                    op1=mybir.AluOpType.subtract,
                )

            # ---- persistent y (bf16) and stats accumulators (PSUM) ----
            y_all = persist.tile([128, BT * D_OUT], BF16)    # [b%128, (bt, o)]
            sy = spsum_pool.tile([1, D_OUT], F32)            # sum(y)   per o
            sy2 = spsum_pool.tile([1, D_OUT], F32)           # sum(y^2) per o

            # ---- main loop over batch tiles ----
            # stats matmuls run one bt behind the main matmuls so the tensor
            # engine never waits on the DVE/ACT producers of their inputs.
            pending_stats = []

            def flush_stats(first, last):
                for ot, yslice, y2t in pending_stats:
                    nc.tensor.matmul(sy[0:1, ot * 512:ot * 512 + 512],
                                     ones[:], yslice,
                                     start=first, stop=last)
                    nc.tensor.matmul(sy2[0:1, ot * 512:ot * 512 + 512],
                                     ones[:], y2t[:],
                                     start=first, stop=last)
                pending_stats.clear()

            for bt in range(BT):
                if bt == 0:
                    xb = xb0
                else:
                    xb = xin_pool.tile([128, KT * 128], BF16, name=f"xb{bt}",
                                       tag="xb")
                    nc.gpsimd.dma_start(
                        xb[:], xt[bt * 128:(bt + 1) * 128, :])
                new_stats = []
                for ot in range(2):
                    acc = psum_pool.tile([128, 512], F32, name=f"acc{bt}_{ot}",
                                         tag=f"acc{ot}")
                    for it in range(KT):
                        nc.tensor.matmul(
                            acc[:],
                            xb[:, it * 128:(it + 1) * 128],
                            wbs[it][:, ot * 512:ot * 512 + 512],
                            start=(it == 0), stop=(it == KT - 1),
                        )
                    yslice = y_all[:, bt * D_OUT + ot * 512:
                                   bt * D_OUT + ot * 512 + 512]
                    nc.vector.tensor_copy(yslice, acc[:])
                    y2t = work_pool.tile([128, 512], BF16, name=f"y2_{bt}_{ot}",
                                         tag=f"y2_{ot}")
                    nc.scalar.activation(y2t[:], acc[:],
                                         mybir.ActivationFunctionType.Square)
                    new_stats.append((ot, yslice, y2t))
                flush_stats(bt == 1 or bt == 13, bt == 12)
                if bt == 12:
                    # part-A stats (bts 0-11) all-reduce, overlapped under the
                    # remaining matmuls; psum accumulators are reused for B
                    stats_a = persist.tile([1, 2 * D_OUT], F32)
                    nc.vector.tensor_copy(stats_a[0:1, 0:D_OUT], sy[:])
                    nc.vector.tensor_copy(stats_a[0:1, D_OUT:2 * D_OUT],
                                          sy2[:])
                    cbi_a = dram.tile([1, 2 * D_OUT], F32)
                    cbo_a = dram.tile([1, 2 * D_OUT], F32)
                    nc.gpsimd.dma_start(cbi_a[:], stats_a[:])
                    nc.gpsimd.collective_compute(
                        "AllReduce", mybir.AluOpType.add,
                        replica_groups=[list(range(N_CORES))],
                        ins=[cbi_a.opt()], outs=[cbo_a.opt()],
                    )
                pending_stats.extend(new_stats)

                if AR_WARM and bt % AR_WARM == AR_WARM - 1 and bt < BT - 1:
                    # Paced dummy all-reduces keep the collective engine awake
                    # so the real stats all-reduce is cheap.  Nothing ever
                    # waits on their outputs.
                    wi = dram.tile([1, 8], F32, name=f"warm_i{bt}",
                                   tag=f"warm_i{bt}")
                    wo = dram.tile([1, 8], F32, name=f"warm_o{bt}",
                                   tag=f"warm_o{bt}")
                    nc.gpsimd.dma_start(
                        wi[:], y_all[0:1, max(bt - 2, 0) * D_OUT:
                                     max(bt - 2, 0) * D_OUT + 8])
                    nc.gpsimd.collective_compute(
                        "AllReduce", mybir.AluOpType.add,
                        replica_groups=[list(range(N_CORES))],
                        ins=[wi.opt()], outs=[wo.opt()],
                    )
            flush_stats(False, True)

            # ---- global stats all-reduce ----
            stats = persist.tile([1, 2 * D_OUT], F32)
            nc.vector.tensor_copy(stats[0:1, 0:D_OUT], sy[:])
            nc.vector.tensor_copy(stats[0:1, D_OUT:2 * D_OUT], sy2[:])
            cbi = dram.tile([1, 2 * D_OUT], F32)
            cbo = dram.tile([1, 2 * D_OUT], F32)
            nc.gpsimd.dma_start(cbi[:], stats[:])
            nc.gpsimd.collective_compute(
                "AllReduce", mybir.AluOpType.add,
                replica_groups=[list(range(N_CORES))],
                ins=[cbi.opt()], outs=[cbo.opt()],
            )
            # coefficient math in [128, 8] layout (o = p*8 + j) so all 128
            # DVE lanes work instead of one
            PJ = 2 * D_OUT // 128     # 16 = [a-half 8 | c-half 8]
            gs8 = persist.tile([128, PJ], F32)
            gs8a = persist.tile([128, PJ], F32)
            nc.sync.dma_start(gs8a[:, 0:8], cbo_a[0:1, 0:D_OUT].rearrange(
                "a (p j) -> (a p) j", p=128))
            nc.sync.dma_start(gs8a[:, 8:16], cbo_a[0:1, D_OUT:2 * D_OUT]
                              .rearrange("a (p j) -> (a p) j", p=128))
            nc.sync.dma_start(gs8[:, 0:8], cbo[0:1, 0:D_OUT].rearrange(
                "a (p j) -> (a p) j", p=128))
            nc.sync.dma_start(gs8[:, 8:16], cbo[0:1, D_OUT:2 * D_OUT].rearrange(
                "a (p j) -> (a p) j", p=128))
            nc.vector.tensor_tensor(out=gs8[:], in0=gs8[:], in1=gs8a[:],
                                    op=mybir.AluOpType.add)

            # ---- coefficients: a = gamma/sqrt(var+eps/4), c = beta - mean*a
            gam = persist.tile([128, 8], F32)
            bet = persist.tile([128, 8], F32)
            nc.sync.dma_start(gam[:], gamma.rearrange("a (p j) -> (a p) j",
                                                      p=128))
            nc.sync.dma_start(bet[:], beta.rearrange("a (p j) -> (a p) j",
                                                     p=128))

            coef = persist.tile([128, PJ], F32)   # [a(8) | c(8)] per partition
            mean = persist.tile([128, 8], F32)
            var = persist.tile([128, 8], F32)
            m2 = persist.tile([128, 8], F32)
            inv = persist.tile([128, 8], F32)
            nc.vector.tensor_scalar_mul(mean[:], gs8[:, 0:8], 1.0 / B_TOT)
            nc.vector.tensor_scalar_mul(var[:], gs8[:, 8:16], 1.0 / B_TOT)
            nc.vector.tensor_tensor(out=m2[:], in0=mean[:], in1=mean[:],
                                    op=mybir.AluOpType.mult)
            nc.vector.tensor_tensor(out=var[:], in0=var[:], in1=m2[:],
                                    op=mybir.AluOpType.subtract)
            nc.vector.tensor_scalar_add(var[:], var[:], BN_EPS / 4.0)
            nc.scalar.activation(inv[:], var[:],
                                 mybir.ActivationFunctionType.Sqrt)
            nc.vector.reciprocal(inv[:], inv[:])
            nc.vector.tensor_tensor(out=coef[:, 0:8], in0=gam[:],
                                    in1=inv[:], op=mybir.AluOpType.mult)
            # c = beta - mean * a
            tmp_ma = persist.tile([128, 8], F32)
            nc.vector.tensor_tensor(out=tmp_ma[:], in0=mean[:],
                                    in1=coef[:, 0:8],
                                    op=mybir.AluOpType.mult)
            nc.vector.tensor_tensor(out=coef[:, 8:16],
                                    in0=bet[:], in1=tmp_ma[:],
                                    op=mybir.AluOpType.subtract)

            # ---- broadcast coefficients to all 128 partitions ----
            # coef[p, 0:8] holds a[p*8 : p*8+8]; write back to DRAM flat,
            # then broadcast-read
            coefd = dram.tile([1, 2 * D_OUT], F32)
            nc.sync.dma_start(coefd[0:1, :].rearrange(
                "a (half p j) -> (a p) half j", p=128, half=2), coef[:]
                .rearrange("p (half j) -> p half j", half=2))
            REP = 4                                   # bt blocks per DVE op
            ab = persist.tile([128, D_OUT], BF16)
            cb = persist.tile([128, D_OUT], BF16)
            nc.gpsimd.dma_start(ab[:], coefd[0:1, 0:D_OUT]
                                .partition_broadcast(128))
            nc.gpsimd.dma_start(cb[:], coefd[0:1, D_OUT:2 * D_OUT]
                                .partition_broadcast(128))

            # ---- normalize and write out (all bf16 for DVE 2x mode;
            # ---- the store DMA casts bf16 -> f32) ----
            for c in range(BT // REP):
                w_ = REP * D_OUT
                tmp = work_pool.tile([128, w_], BF16, name=f"nt{c}",
                                     tag="ntmp")
                nc.vector.tensor_tensor(
                    out=tmp[:].rearrange("p (r o) -> p r o", r=REP),
                    in0=y_all[:, c * w_:(c + 1) * w_]
                    .rearrange("p (r o) -> p r o", r=REP),
                    in1=ab[:].unsqueeze(1).broadcast_to((128, REP, D_OUT)),
                    op=mybir.AluOpType.mult)
                stg = stage_pool.tile([128, w_], BF16, name=f"stg{c}",
                                      tag="stg")
                nc.vector.tensor_tensor(
                    out=stg[:].rearrange("p (r o) -> p r o", r=REP),
                    in0=tmp[:].rearrange("p (r o) -> p r o", r=REP),
                    in1=cb[:].unsqueeze(1).broadcast_to((128, REP, D_OUT)),
                    op=mybir.AluOpType.add)
                nc.gpsimd.dma_start(
                    out.rearrange("(c r p) o -> c p r o",
                                  r=REP, p=128)[c, :, :, :],
                    stg[:].rearrange("p (r o) -> p r o", r=REP))

    nc.compile()
    return nc


_NC_CACHE = None


def kernel(x, weight, bias, gamma, beta):
    global _NC_CACHE
    if _NC_CACHE is None:
        _NC_CACHE = build_kernel()
    nc = _NC_CACHE

    x = np.asarray(x, dtype=np.float32)
    weight = np.asarray(weight, dtype=np.float32)
    gamma = np.asarray(gamma, dtype=np.float32).reshape(1, D_OUT)
    beta = np.asarray(beta, dtype=np.float32).reshape(1, D_OUT)

    wt = np.ascontiguousarray(weight.T)
    in_maps = []
    for i in range(N_CORES):
        shard = x[i * B_SH:(i + 1) * B_SH]
        blk = shard.reshape(BT, 128, KT, 128).transpose(0, 3, 2, 1)
        in_maps.append({
            "xt": np.ascontiguousarray(blk).reshape(BT * 128, KT * 128),
            "wt": wt,
            "gamma": gamma,
            "beta": beta,
        })

    res = bass_utils.run_bass_kernel_spmd(
        nc, in_maps, core_ids=list(range(N_CORES)),
        trace=bool(int(os.environ.get("KERNEL_TRACE", "0"))),
    )
    kernel.last_results = res
    return np.concatenate([res.results[i]["out"] for i in range(N_CORES)],
                          axis=0)


# revision 26
# speedup vs baseline: 1.1590x; 1.1071x over previous
"""Trainium2 SPMD kernel for: y = BatchNorm1d(x @ sign(w).T + bias) * gamma + beta.

Sharding: data-parallel over the batch dim across 8 NeuronCores; the
weight is replicated.  BatchNorm batch statistics are produced with an
on-device AllReduce of per-shard (sum_y, sum_y2).

Math notes:
  - The linear bias cancels inside BatchNorm (y - mean), so it is never
    applied on device.
  - sign(w) in {-1,+1} is computed as (w >= 0) - 0.5 in {-0.5,+0.5}; the
    resulting global scale of 0.5 also cancels in BatchNorm except in the
    epsilon, which is compensated with eps/4.
  - Matmul runs in bf16 (weights +-0.5 are exact; x rounding gives
    ~2e-3 relative error, far inside tolerance).  fp32->bf16 conversion
    happens inside the DMA (SWDGE casting DMA), not on compute engines.
"""

import os
import sys

sys.path.insert(0, "/opt/trn_rl_repo")

import numpy as np

import concourse.bacc as bacc
import concourse.mybir as mybir
import concourse.tile as tile
from concourse import bass_utils

N_CORES = 8
B_TOT = 16384
D_IN = 2048
D_OUT = 1024
B_SH = B_TOT // N_CORES           # 2048 rows per core
KT = D_IN // 128                  # 16 contraction tiles
BT = B_SH // 128                  # 16 batch tiles per core
BN_EPS = 1e-5

F32 = mybir.dt.float32
BF16 = mybir.dt.bfloat16

# every AR_WARM-th batch tile fires a dummy all-reduce; 0 disables
AR_WARM = int(os.environ.get("KERNEL_AR_WARM", "3"))


def build_kernel():
    nc = bacc.Bacc("TRN2", target_bir_lowering=False, debug=False,
                   num_devices=N_CORES)

    xt = nc.dram_tensor("xt", [D_IN, B_SH], F32, kind="ExternalInput")
    wt = nc.dram_tensor("wt", [D_IN, D_OUT], F32, kind="ExternalInput")
    gamma = nc.dram_tensor("gamma", [1, D_OUT], F32, kind="ExternalInput")
    beta = nc.dram_tensor("beta", [1, D_OUT], F32, kind="ExternalInput")
    out = nc.dram_tensor("out", [B_SH, D_OUT], F32, kind="ExternalOutput")

    with tile.TileContext(nc) as tc:
        with tc.tile_pool(name="persist", bufs=1) as persist, \
             tc.tile_pool(name="xin", bufs=4) as xin_pool, \
             tc.tile_pool(name="work", bufs=3) as work_pool, \
             tc.tile_pool(name="stage", bufs=3) as stage_pool, \
             tc.tile_pool(name="psum", bufs=2, space="PSUM") as psum_pool, \
             tc.tile_pool(name="spsum", bufs=1, space="PSUM") as spsum_pool, \
             tc.tile_pool(name="dram", bufs=1, space="DRAM") as dram:

            ones = persist.tile([128, 1], BF16)
            nc.vector.memset(ones[:], 1.0)

            # ---- weights: per-stripe tiles so matmuls depend only on their
            # ---- own stripe; loads split across both HWDGE rails ----
            wbs = [persist.tile([128, D_OUT], BF16, name=f"wb{i}")
                   for i in range(KT)]
            xb0 = xin_pool.tile([128, KT * 128], BF16, name="xb0", tag="xb")
            # start the first x block as early as possible (host supplies
            # xt in bt-major blocks: row bt*128+p, col kt*128+b -> the load
            # is a plain contiguous 2D slice)
            nc.gpsimd.dma_start(xb0[:], xt[0:128, :])
            for it in range(KT):
                # HWDGE f32 load; the sign op below does the bf16 conversion
                wtmp = work_pool.tile([128, D_OUT], F32, name=f"wtmp{it}",
                                      tag=f"wtmp{it % 4}")
                eng = nc.sync if it % 2 == 0 else nc.scalar
                eng.dma_start(wtmp[:], wt[it * 128:(it + 1) * 128, :])
                # (w >= 0) - 0.5  ->  {+0.5, -0.5}
                nc.vector.tensor_scalar(
                    out=wbs[it][:],
                    in0=wtmp[:],
                    scalar1=0.0, scalar2=0.5,
                    op0=mybir.AluOpType.is_ge,
                    op1=mybir.AluOpType.subtract,
                )

            # ---- persistent y (bf16) and stats accumulators (PSUM) ----
            y_all = persist.tile([128, BT * D_OUT], BF16)    # [b%128, (bt, o)]
            sy = spsum_pool.tile([1, D_OUT], F32)            # sum(y)   per o
            sy2 = spsum_pool.tile([1, D_OUT], F32)           # sum(y^2) per o

            # ---- main loop over batch tiles ----
            # stats matmuls run one bt behind the main matmuls so the tensor
            # engine never waits on the DVE/ACT producers of their inputs.
            pending_stats = []

            def flush_stats(first, last):
                for ot, yslice, y2t in pending_stats:
                    nc.tensor.matmul(sy[0:1, ot * 512:ot * 512 + 512],
                                     ones[:], yslice,
                                     start=first, stop=last)
                    nc.tensor.matmul(sy2[0:1, ot * 512:ot * 512 + 512],
                                     ones[:], y2t[:],
                                     start=first, stop=last)
                pending_stats.clear()

            for bt in range(BT):
                if bt == 0:
                    xb = xb0
                else:
                    xb = xin_pool.tile([128, KT * 128], BF16, name=f"xb{bt}",
                                       tag="xb")
                    nc.gpsimd.dma_start(
                        xb[:], xt[bt * 128:(bt + 1) * 128, :])
                new_stats = []
                for ot in range(2):
                    acc = psum_pool.tile([128, 512], F32, name=f"acc{bt}_{ot}",
                                         tag=f"acc{ot}")
                    for it in range(KT):
                        nc.tensor.matmul(
                            acc[:],
                            xb[:, it * 128:(it + 1) * 128],
                            wbs[it][:, ot * 512:ot * 512 + 512],
                            start=(it == 0), stop=(it == KT - 1),
                        )
                    yslice = y_all[:, bt * D_OUT + ot * 512:
                                   bt * D_OUT + ot * 512 + 512]
                    nc.vector.tensor_copy(yslice, acc[:])
                    y2t = work_pool.tile([128, 512], BF16, name=f"y2_{bt}_{ot}",
                                         tag=f"y2_{ot}")
                    nc.scalar.activation(y2t[:], acc[:],
                                         mybir.ActivationFunctionType.Square)
                    new_stats.append((ot, yslice, y2t))
                flush_stats(bt == 1, False)
                pending_stats.extend(new_stats)

                if AR_WARM and bt % AR_WARM == AR_WARM - 1 and bt < BT - 1:
                    # Paced dummy all-reduces keep the collective engine awake
                    # so the real stats all-reduce is cheap.  Nothing ever
                    # waits on their outputs.
                    wi = dram.tile([1, 8], F32, name=f"warm_i{bt}",
                                   tag=f"warm_i{bt}")
                    wo = dram.tile([1, 8], F32, name=f"warm_o{bt}",
                                   tag=f"warm_o{bt}")
                    nc.gpsimd.dma_start(
                        wi[:], y_all[0:1, max(bt - 2, 0) * D_OUT:
                                     max(bt - 2, 0) * D_OUT + 8])
                    nc.gpsimd.collective_compute(
                        "AllReduce", mybir.AluOpType.add,
                        replica_groups=[list(range(N_CORES))],
                        ins=[wi.opt()], outs=[wo.opt()],
                    )
            flush_stats(False, True)

            # ---- global stats all-reduce ----
            stats = persist.tile([1, 2 * D_OUT], F32)
            nc.vector.tensor_copy(stats[0:1, 0:D_OUT], sy[:])
            nc.vector.tensor_copy(stats[0:1, D_OUT:2 * D_OUT], sy2[:])
            cbi = dram.tile([1, 2 * D_OUT], F32)
            cbo = dram.tile([1, 2 * D_OUT], F32)
            nc.gpsimd.dma_start(cbi[:], stats[:])
            nc.gpsimd.collective_compute(
                "AllReduce", mybir.AluOpType.add,
                replica_groups=[list(range(N_CORES))],
                ins=[cbi.opt()], outs=[cbo.opt()],
            )
            # coefficient math in [128, 8] layout (o = p*8 + j) so all 128
            # DVE lanes work instead of one
            PJ = 2 * D_OUT // 128     # 16 = [a-half 8 | c-half 8]
            gs8 = persist.tile([128, PJ], F32)
            nc.sync.dma_start(gs8[:, 0:8], cbo[0:1, 0:D_OUT].rearrange(
                "a (p j) -> (a p) j", p=128))
            nc.sync.dma_start(gs8[:, 8:16], cbo[0:1, D_OUT:2 * D_OUT].rearrange(
                "a (p j) -> (a p) j", p=128))

            # ---- coefficients: a = gamma/sqrt(var+eps/4), c = beta - mean*a
            gam = persist.tile([128, 8], F32)
            bet = persist.tile([128, 8], F32)
            nc.sync.dma_start(gam[:], gamma.rearrange("a (p j) -> (a p) j",
                                                      p=128))
            nc.sync.dma_start(bet[:], beta.rearrange("a (p j) -> (a p) j",
                                                     p=128))

            coef = persist.tile([128, PJ], F32)   # [a(8) | c(8)] per partition
            mean = persist.tile([128, 8], F32)
            var = persist.tile([128, 8], F32)
            m2 = persist.tile([128, 8], F32)
            inv = persist.tile([128, 8], F32)
            nc.vector.tensor_scalar_mul(mean[:], gs8[:, 0:8], 1.0 / B_TOT)
            nc.vector.tensor_scalar_mul(var[:], gs8[:, 8:16], 1.0 / B_TOT)
            nc.vector.tensor_tensor(out=m2[:], in0=mean[:], in1=mean[:],
                                    op=mybir.AluOpType.mult)
            nc.vector.tensor_tensor(out=var[:], in0=var[:], in1=m2[:],
                                    op=mybir.AluOpType.subtract)
            nc.vector.tensor_scalar_add(var[:], var[:], BN_EPS / 4.0)
            nc.scalar.activation(inv[:], var[:],
                                 mybir.ActivationFunctionType.Sqrt)
            nc.vector.reciprocal(inv[:], inv[:])
            nc.vector.tensor_tensor(out=coef[:, 0:8], in0=gam[:],
                                    in1=inv[:], op=mybir.AluOpType.mult)
            # c = beta - mean * a
            tmp_ma = persist.tile([128, 8], F32)
            nc.vector.tensor_tensor(out=tmp_ma[:], in0=mean[:],
                                    in1=coef[:, 0:8],
                                    op=mybir.AluOpType.mult)
            nc.vector.tensor_tensor(out=coef[:, 8:16],
                                    in0=bet[:], in1=tmp_ma[:],
                                    op=mybir.AluOpType.subtract)

            # ---- broadcast coefficients to all 128 partitions ----
            # coef[p, 0:8] holds a[p*8 : p*8+8]; write back to DRAM flat,
            # then broadcast-read
            coefd = dram.tile([1, 2 * D_OUT], F32)
            nc.sync.dma_start(coefd[0:1, :].rearrange(
                "a (half p j) -> (a p) half j", p=128, half=2), coef[:]
                .rearrange("p (half j) -> p half j", half=2))
            REP = 4                                   # bt blocks per DVE op
            ab = persist.tile([128, D_OUT], BF16)
            cb = persist.tile([128, D_OUT], BF16)
            nc.gpsimd.dma_start(ab[:], coefd[0:1, 0:D_OUT]
                                .partition_broadcast(128))
            nc.gpsimd.dma_start(cb[:], coefd[0:1, D_OUT:2 * D_OUT]
                                .partition_broadcast(128))

            # ---- normalize and write out (all bf16 for DVE 2x mode;
            # ---- the store DMA casts bf16 -> f32) ----
            for c in range(BT // REP):
                w_ = REP * D_OUT
                tmp = work_pool.tile([128, w_], BF16, name=f"nt{c}",
                                     tag="ntmp")
                nc.vector.tensor_tensor(
                    out=tmp[:].rearrange("p (r o) -> p r o", r=REP),
                    in0=y_all[:, c * w_:(c + 1) * w_]
                    .rearrange("p (r o) -> p r o", r=REP),
                    in1=ab[:].unsqueeze(1).broadcast_to((128, REP, D_OUT)),
                    op=mybir.AluOpType.mult)
                stg = stage_pool.tile([128, w_], BF16, name=f"stg{c}",
                                      tag="stg")
                nc.vector.tensor_tensor(
                    out=stg[:].rearrange("p (r o) -> p r o", r=REP),
                    in0=tmp[:].rearrange("p (r o) -> p r o", r=REP),
                    in1=cb[:].unsqueeze(1).broadcast_to((128, REP, D_OUT)),
                    op=mybir.AluOpType.add)
                nc.gpsimd.dma_start(
                    out.rearrange("(c r p) o -> c p r o",
                                  r=REP, p=128)[c, :, :, :],
                    stg[:].rearrange("p (r o) -> p r o", r=REP))

    nc.compile()
    return nc


_NC_CACHE = None


def kernel(x, weight, bias, gamma, beta):
    global _NC_CACHE
    if _NC_CACHE is None:
        _NC_CACHE = build_kernel()
    nc = _NC_CACHE

    x = np.asarray(x, dtype=np.float32)
    weight = np.asarray(weight, dtype=np.float32)
    gamma = np.asarray(gamma, dtype=np.float32).reshape(1, D_OUT)
    beta = np.asarray(beta, dtype=np.float32).reshape(1, D_OUT)

    wt = np.ascontiguousarray(weight.T)
    in_maps = []
    for i in range(N_CORES):
        shard = x[i * B_SH:(i + 1) * B_SH]
        blk = shard.reshape(BT, 128, KT, 128).transpose(0, 3, 2, 1)
        in_maps.append({
            "xt": np.ascontiguousarray(blk).reshape(BT * 128, KT * 128),
            "wt": wt,
            "gamma": gamma,
            "beta": beta,
        })

    res = bass_utils.run_bass_kernel_spmd(
        nc, in_maps, core_ids=list(range(N_CORES)),
        trace=bool(int(os.environ.get("KERNEL_TRACE", "0"))),
    )
    kernel.last_results = res
    return np.concatenate([res.results[i]["out"] for i in range(N_CORES)],
                          axis=0)
